# revision 34
# baseline (speedup 1.0000x reference)
"""Trainium2 Bass kernel for the chunked quadratic-attention contraction:

    out = 0.5 * einsum('bhndef,bhncd,bhnce->bhncf', S, Qc, Qc),  Qc = (q/8) chunked

Strategy
--------
out[c,f] = sum_{d,e} Qc[c,d] Qc[c,e] S[d,e,f] is a quadratic form per row.
The host expands it into a plain matmul over packed (d<=e) pairs:

    G2[c, p]   = 0.5 * Qc[c, d_p] * Qc[c, e_p]          (p = packed pair d<=e, 2080 pairs)
    Ssym[p, f] = S[d_p, e_p, f] + S[e_p, d_p, f]        (halved on the diagonal)
    out[c, f]  = sum_p G2[c, p] * Ssym[p, f]

Both operands ship as fp8 e3m4 (G2 x78, Ssym x2; the output copy divides by
156) and the output as fp16 (K split as 16 full 128-tiles + one 32-row
tail). Per (b,h) head — one head per NeuronCore, 8 cores — the device runs
8 block-pairs of two 17-step PSUM-accumulating matmul chains (K<=128, M=64,
N=256) that execute concurrently in the PE's two column groups.

Schedule — what the ~43 us is made of
-------------------------------------
~11.1 MB/core must stream HBM->SBUF through the 16 DMA engines at
~360-400 GB/s: a hard ~28 us floor that dominates everything. All inputs
ride the sync HWDGE queue as ~14 large DMAs of host-packed blobs
(K-major cells [Ssym_k 64 B | G2_k 256 B], chains interleaved per K, so
a K-range DMA split feeds both matmul chains of a pair).

The PE chews a block-pair in ~1.9 us vs ~3.2 us to stream one, and any
multi-us idle drops its p-state clock (2.4 -> 1.2 GHz; ~3 us of
continuous work to ramp back), so a stream-paced PE stalls cold at every
pair boundary and runs the endgame at half clock. Instead, pair-0's
blobs (h0/h1) ride BEHIND g1/g2 in the stream: the PE's first matmul is
gated on h1, it starts with ~2 pairs of lookahead resident, and then
runs all 8 pairs as ONE continuous 100%-busy warm burst (~18.7 us) that
ends just past the stream end. The last pair streams in descending
pieces ([3,3,1,1]/8) and runs its K-tail tile (long-resident gst) first.
Output flushes ride the scalar (Act) HWDGE queue, triggered per-group as
the PSUM->SBUF copies complete; only the final 64 KB flush trails the
last pair's copy.

Measured dead ends (all worse): anchoring the PE earlier (stream jitter
turns the thinner lookahead into long cold stalls), finer trailing
pieces (extra issue/semaphore churn), and p-state warm-up dummies in any
form (lone or chained dummy matmul groups do not overlap column groups
the way the real interleaved accumulation chains do). Do NOT wrap the
DMA block in tc.high_priority(): it resets the priority counter, later
instructions get interleaving priorities, and the tile scheduler
scrambles the queue order the whole schedule depends on.
"""

import sys
import numpy as np

for _p in ("/opt/trn_rl_repo", "/root/.axon_site/_ro/trn_rl_repo"):
    if _p not in sys.path:
        sys.path.insert(0, _p)

B, H, S_LEN, D = 1, 8, 4096, 64
N_CHUNK = 16          # sequence chunks per head
C = 256               # rows per chunk
PAIRS = (D * (D + 1)) // 2   # 2080 packed (d<=e) pairs
KFULL = 16            # full 128-row K tiles
KTAIL = PAIRS - KFULL * 128  # 32
KTILES = KFULL + 1    # 17
N_CORES = 8
NPAIR = N_CHUNK // 2  # 8 block pairs

_iu, _ju = np.triu_indices(D)
_wsym = np.where(_iu == _ju, 0.5, 1.0).astype(np.float32)

# fp8 e3m4 max normal is 15.5; G2 absmax is ~0.2, so x78 fills the range.
# Ssym (absmax ~7.7) ships as e3m4 at x2; the device copy divides by 156.
G_SCALE = 78.0
S_SCALE = 2.0
F8_MAX = 15.5

KSTRIDE = D + C               # bytes per (chain, K-tile) cell: [Ssym | G2]
HB = KFULL * KSTRIDE          # head blob (one chain)  = 5120 B/partition
GB = 2 * KFULL * KSTRIDE      # group blob (two chains) = 10240 B/partition

_compiled = None


def _build_module():
    import concourse.mybir as mybir
    import concourse.tile as tile
    from concourse import bacc

    f8 = mybir.dt.float8e3
    f16 = mybir.dt.float16
    f32 = mybir.dt.float32

    nc = bacc.Bacc("TRN2", target_bir_lowering=False, debug=False)
    # blob[:, :]: [h0 | h1 | grp1..grp7] — pair-0 chain blobs (16 cells of
    # [ssa_k (64) | g0_k (256)] each) followed by the 7 pair-group blobs
    # (32 K-major cells of [ssb (64) | gt (256)] each). One dram tensor
    # keeps the runtime's tensor/descriptor init (preamble) minimal.
    blob = nc.dram_tensor("blob", [128, 2 * HB + (NPAIR - 1) * GB], f8,
                          kind="ExternalInput")
    hb = [blob[:, 0:HB], blob[:, HB : 2 * HB]]
    grp = [
        blob[:, 2 * HB + (j - 1) * GB : 2 * HB + j * GB] for j in range(1, NPAIR)
    ]
    # gst[pp, :]: K-tail rows (pp < 32): [sst (n,f) 1024 B | gtta (j,i,c) 4096 B]
    GSTW = N_CHUNK * D + NPAIR * 2 * C
    gst = nc.dram_tensor("gst", [KTAIL, GSTW], f8, kind="ExternalInput")
    # outd[q, n2, c]: q = f + 64*i for block n = 2*n2+i
    outd = nc.dram_tensor("out", [128, NPAIR, C], f16, kind="ExternalOutput")

    with tile.TileContext(nc) as tc:
        with (
            tc.tile_pool(name="blob_pool", bufs=1) as bp,
            tc.tile_pool(name="psum", bufs=4, space="PSUM") as pp,
            tc.tile_pool(name="osb_pool", bufs=3) as op,
        ):
            # Single input queue (sync). Stream order is NOT consumption
            # order: pair-0's blobs (h0/h1) ride BEHIND g1/g2 so the PE's
            # first matmul is gated until ~2 pairs of lookahead are
            # resident. The PE (2x faster than the stream, and penalized
            # ~2x for ~3us after any long idle by the p-state ramp) then
            # runs all 8 pairs in ONE continuous warm burst that finishes
            # right as the last group piece lands, instead of repeatedly
            # stalling cold at pair boundaries.
            # NOTE: no high_priority() here — it resets the priority
            # counter, so instructions created after the block would get
            # priorities interleaving with those inside it and the tile
            # scheduler would scramble the queue order. Plain program
            # order gives strictly monotone priorities.
            gt_tiles = {}
            gstt = bp.tile([KTAIL, GSTW], f8, tag="gst")
            nc.sync.dma_start(out=gstt[:], in_=gst[:])
            g1 = bp.tile([128, GB], f8, tag="grp1")
            nc.sync.dma_start(out=g1[:], in_=grp[0])
            gt_tiles[1] = g1
            g2 = bp.tile([128, GB], f8, tag="grp2")
            nc.sync.dma_start(out=g2[:], in_=grp[1])
            gt_tiles[2] = g2
            h0 = bp.tile([128, HB], f8, tag="h0")
            nc.sync.dma_start(out=h0[:], in_=hb[0])
            h1 = bp.tile([128, HB], f8, tag="h1")
            nc.sync.dma_start(out=h1[:], in_=hb[1])
            stt = gstt[:, : N_CHUNK * D]
            gta = gstt[:, N_CHUNK * D :]

            # Later groups stream whole (g3-g5), in halves (g6), and in
            # descending pieces (g7) so the PE's trailing granule at the
            # stream end is minimal.
            for j in range(3, NPAIR):
                assert j not in gt_tiles
                g = bp.tile([128, GB], f8, tag=f"grp{j}")
                if j == NPAIR - 1:
                    pieces = [3, 3, 1, 1]
                elif j == NPAIR - 2:
                    pieces = [4, 4]
                else:
                    pieces = [8]
                off = 0
                u = GB // 8
                for np_ in pieces:
                    nc.sync.dma_start(
                        out=g[:, off : off + np_ * u],
                        in_=grp[j - 1][:, off : off + np_ * u],
                    )
                    off += np_ * u
                gt_tiles[j] = g

            # PE execution order [2, 1, 0, 3..7]: pair 2 is gated on g2
            # (resident ~3 us before h1), so the p-state cold-start ramp is
            # spent on real work from the lookahead buffer and pair 0 runs
            # third, after h1 has arrived — the burst starts ~3 us earlier
            # for free. Stream order is unchanged.
            osb_tiles = {}
            osb = None
            gs = 0
            flush_at = {3: (0, 4), 6: (4, 3), 7: (7, 1)}
            flushes = []
            for j in (2, 1, 0, 3, 4, 5, 6, 7):
                gs = 0 if j < 4 else (4 if j < 7 else 7)
                if gs not in osb_tiles:
                    osb_tiles[gs] = op.tile([128, 4, C], f16, name=f"osb{gs}", tag=f"osb{gs}")
                osb = osb_tiles[gs]
                ps = pp.tile([128, C], f32)
                # All pairs interleave both chains (running a chain solo
                # halves PE throughput — the column groups only overlap
                # when the instruction stream alternates them). The last
                # pair runs the K-tail tile (stt/gta, long since resident)
                # FIRST so the final matmuls after the last grp piece lands
                # are plain 128-row tiles, not the slow 32-row tail. Pair 0
                # leads with chain 1 (h1, the later-arriving blob) so the
                # PE's continuous burst starts only once both are there.
                if j == NPAIR - 1:
                    korder = [KTILES - 1] + list(range(KFULL))
                    ki = [(k, i) for k in korder for i in range(2)]
                elif j == 0:
                    ki = [(k, i) for k in range(KTILES) for i in (1, 0)]
                else:
                    ki = [(k, i) for k in range(KTILES) for i in range(2)]
                kfirst, klast = ki[0][0], ki[-1][0]
                for k, i in ki:
                    n = 2 * j + i
                    if k < KFULL:
                        if j == 0:
                            blob, base = (h0 if i == 0 else h1), k * KSTRIDE
                        else:
                            blob = gt_tiles[j]
                            base = (2 * k + i) * KSTRIDE
                        lhsT = blob[:, base : base + D]
                        rhs = blob[:, base + D : base + D + C]
                    else:
                        lhsT = stt[:, n * D : n * D + D]
                        rhs = gta[:, (j * 2 + i) * C : (j * 2 + i) * C + C]
                    nc.tensor.matmul(
                        ps[64 * i : 64 * i + 64, :],
                        lhsT=lhsT,
                        rhs=rhs,
                        start=(k == kfirst),
                        stop=(k == klast),
                        tile_position=(0, 64 * i),
                    )
                scale = 1.0 / (G_SCALE * S_SCALE)
                nc.vector.tensor_scalar_mul(
                    out=osb[:, j - gs, :], in0=ps[:], scalar1=scale
                )
                if j in flush_at:
                    j0, cnt = flush_at[j]
                    flushes.append((outd[:, j0 : j0 + cnt, :], osb[:, :cnt, :]))
            # Output flushes ride the scalar (Act) HWDGE queue: issued as
            # soon as their osb tiles are ready, they overlap the input
            # stream (byte-count is conserved either way) and keep the
            # final 64 KB flush off the end of the deep sync queue.
            for dst, src in flushes:
                nc.scalar.dma_start(out=dst, in_=src)
    nc.finalize()
    return nc


def _get_compiled():
    global _compiled
    if _compiled is None:
        _compiled = _build_module()
    return _compiled


def _host_prepare(q, kv_quad_state):
    import ml_dtypes

    f8 = ml_dtypes.float8_e3m4
    qc = (q[0].astype(np.float32) * (D ** -0.5)).reshape(H, N_CHUNK, C, D)
    kv = kv_quad_state[0].astype(np.float32)  # (H, N, D, D, D)
    in_maps = []
    for h in range(H):
        # --- G2 (moving operand, transposed to K-major) ---
        G = qc[h][:, :, _iu] * qc[h][:, :, _ju]          # (N, C, PAIRS)
        G *= 0.5 * G_SCALE
        G8 = np.clip(G, -F8_MAX, F8_MAX).astype(f8)
        # [n, c, kk, pp] -> [n, pp, kk, c]
        gt_dev = (
            G8[:, :, : KFULL * 128]
            .reshape(N_CHUNK, C, KFULL, 128)
            .transpose(0, 3, 2, 1)
        )
        # tail pairs 2048+: [n, c, pp] -> [pp, (j, i, c)]
        gtta_dev = np.ascontiguousarray(
            G8[:, :, KFULL * 128 :].reshape(NPAIR, 2, C, KTAIL).transpose(3, 0, 1, 2)
        ).reshape(KTAIL, NPAIR * 2 * C)
        # --- Ssym (stationary operand, fp8 e3m4 at x2) ---
        Sh = kv[h]                                        # (N, D, D, D)
        Ss = (Sh[:, _iu, _ju, :] + Sh[:, _ju, _iu, :]) * (
            _wsym[None, :, None] * S_SCALE
        )
        Ss8 = np.clip(Ss, -F8_MAX, F8_MAX).astype(f8)     # (N, PAIRS, D)
        # [n, kk, pp, f] -> [n, pp, kk, f]
        ss_dev = (
            Ss8[:, : KFULL * 128, :]
            .reshape(N_CHUNK, KFULL, 128, D)
            .transpose(0, 2, 1, 3)
        )
        # --- blobs: per-partition cells [Ssym_k (64) | G2_k (256)] ---
        cells = np.concatenate([ss_dev, gt_dev], axis=3)  # (N, 128, KFULL, 320)
        hb_dev = cells[:2].reshape(2, 128, HB).transpose(1, 0, 2).reshape(128, 2 * HB)
        # groups: K-major cell pairs [i0_k | i1_k] so K-range DMA splits
        # feed both matmul chains
        grp_dev = (
            cells[2:].reshape(NPAIR - 1, 2, 128, KFULL, KSTRIDE)
            .transpose(2, 0, 3, 1, 4)
            .reshape(128, (NPAIR - 1) * GB)
        )
        blob_dev = np.ascontiguousarray(np.concatenate([hb_dev, grp_dev], axis=1))
        # tail: [n, pp, f] -> [pp, (n, f)]
        sst_dev = np.ascontiguousarray(
            Ss8[:, KFULL * 128 :, :].transpose(1, 0, 2)
        ).reshape(KTAIL, N_CHUNK * D)
        gst_dev = np.concatenate([sst_dev, gtta_dev], axis=1)
        in_maps.append(
            {
                "blob": blob_dev,
                "gst": gst_dev,
            }
        )
    return in_maps


def kernel(q, kv_quad_state, _trace=False):
    from concourse.bass_utils import run_bass_kernel_spmd

    nc = _get_compiled()
    in_maps = _host_prepare(q, kv_quad_state)
    res = run_bass_kernel_spmd(nc, in_maps, core_ids=list(range(N_CORES)), trace=_trace)
    out = np.empty((B, H, S_LEN, D), dtype=np.float32)
    for h in range(H):
        o = res.results[h]["out"].astype(np.float32)      # (128, 8, 256)
        # o[f + 64*i, j, c] = out[block 2j+i, c, f]
        oo = o.reshape(2, D, NPAIR, C).transpose(2, 0, 3, 1)  # (j, i, c, f)
        out[0, h] = oo.reshape(S_LEN, D)
    if _trace:
        kernel.last_exec_time_ns = res.exec_time_ns
        kernel.last_results = res
    return out



# revision 35
# speedup vs baseline: 1.0046x; 1.0046x over previous
"""Trainium2 Bass kernel for the chunked quadratic-attention contraction:

    out = 0.5 * einsum('bhndef,bhncd,bhnce->bhncf', S, Qc, Qc),  Qc = (q/8) chunked

Strategy
--------
out[c,f] = sum_{d,e} Qc[c,d] Qc[c,e] S[d,e,f] is a quadratic form per row.
The host expands it into a plain matmul over packed (d<=e) pairs:

    G2[c, p]   = 0.5 * Qc[c, d_p] * Qc[c, e_p]          (p = packed pair d<=e, 2080 pairs)
    Ssym[p, f] = S[d_p, e_p, f] + S[e_p, d_p, f]        (halved on the diagonal)
    out[c, f]  = sum_p G2[c, p] * Ssym[p, f]

Both operands ship as fp8 e3m4 (G2 x78, Ssym x2; the output copy divides by
156) and the output as fp16 (K split as 16 full 128-tiles + one 32-row
tail). Per (b,h) head — one head per NeuronCore, 8 cores — the device runs
8 block-pairs of two 17-step PSUM-accumulating matmul chains (K<=128, M=64,
N=256) that execute concurrently in the PE's two column groups.

Schedule — what the ~43 us is made of
-------------------------------------
~11.1 MB/core must stream HBM->SBUF through the 16 DMA engines at
~360-400 GB/s: a hard ~28 us floor that dominates everything. All inputs
ride the sync HWDGE queue as ~14 large DMAs of host-packed blobs
(K-major cells [Ssym_k 64 B | G2_k 256 B], chains interleaved per K, so
a K-range DMA split feeds both matmul chains of a pair).

The PE chews a block-pair in ~1.9 us vs ~3.2 us to stream one, and any
multi-us idle drops its p-state clock (2.4 -> 1.2 GHz; ~3 us of
continuous work to ramp back), so a stream-paced PE stalls cold at every
pair boundary and runs the endgame at half clock. Instead, pair-0's
blobs (h0/h1) ride BEHIND g1/g2 in the stream: the PE's first matmul is
gated on h1, it starts with ~2 pairs of lookahead resident, and then
runs all 8 pairs as ONE continuous 100%-busy warm burst (~18.7 us) that
ends just past the stream end. The last pair streams in descending
pieces ([3,3,1,1]/8) and runs its K-tail tile (long-resident gst) first.
Output flushes ride the scalar (Act) HWDGE queue, triggered per-group as
the PSUM->SBUF copies complete; only the final 64 KB flush trails the
last pair's copy.

Measured dead ends (all worse): anchoring the PE earlier (stream jitter
turns the thinner lookahead into long cold stalls), finer trailing
pieces (extra issue/semaphore churn), and p-state warm-up dummies in any
form (lone or chained dummy matmul groups do not overlap column groups
the way the real interleaved accumulation chains do). Do NOT wrap the
DMA block in tc.high_priority(): it resets the priority counter, later
instructions get interleaving priorities, and the tile scheduler
scrambles the queue order the whole schedule depends on.
"""

import sys
import numpy as np

for _p in ("/opt/trn_rl_repo", "/root/.axon_site/_ro/trn_rl_repo"):
    if _p not in sys.path:
        sys.path.insert(0, _p)

B, H, S_LEN, D = 1, 8, 4096, 64
N_CHUNK = 16          # sequence chunks per head
C = 256               # rows per chunk
PAIRS = (D * (D + 1)) // 2   # 2080 packed (d<=e) pairs
KFULL = 16            # full 128-row K tiles
KTAIL = PAIRS - KFULL * 128  # 32
KTILES = KFULL + 1    # 17
N_CORES = 8
NPAIR = N_CHUNK // 2  # 8 block pairs

_iu, _ju = np.triu_indices(D)
_wsym = np.where(_iu == _ju, 0.5, 1.0).astype(np.float32)

# fp8 e3m4 max normal is 15.5; G2 absmax is ~0.2, so x78 fills the range.
# Ssym (absmax ~7.7) ships as e3m4 at x2; the device copy divides by 156.
G_SCALE = 78.0
S_SCALE = 2.0
F8_MAX = 15.5

KSTRIDE = D + C               # bytes per (chain, K-tile) cell: [Ssym | G2]
HB = KFULL * KSTRIDE          # head blob (one chain)  = 5120 B/partition
GB = 2 * KFULL * KSTRIDE      # group blob (two chains) = 10240 B/partition

_compiled = None


def _build_module():
    import concourse.mybir as mybir
    import concourse.tile as tile
    from concourse import bacc

    f8 = mybir.dt.float8e3
    f16 = mybir.dt.float16
    f32 = mybir.dt.float32

    nc = bacc.Bacc("TRN2", target_bir_lowering=False, debug=False)
    # blob[:, :]: [h0 | h1 | grp1..grp7] — pair-0 chain blobs (16 cells of
    # [ssa_k (64) | g0_k (256)] each) followed by the 7 pair-group blobs
    # (32 K-major cells of [ssb (64) | gt (256)] each). One dram tensor
    # keeps the runtime's tensor/descriptor init (preamble) minimal.
    blob = nc.dram_tensor("blob", [128, 2 * HB + (NPAIR - 1) * GB], f8,
                          kind="ExternalInput")
    hb = [blob[:, 0:HB], blob[:, HB : 2 * HB]]
    grp = [
        blob[:, 2 * HB + (j - 1) * GB : 2 * HB + j * GB] for j in range(1, NPAIR)
    ]
    # gst[pp, :]: K-tail rows (pp < 32): [sst (n,f) 1024 B | gtta (j,i,c) 4096 B]
    GSTW = N_CHUNK * D + NPAIR * 2 * C
    gst = nc.dram_tensor("gst", [KTAIL, GSTW], f8, kind="ExternalInput")
    # outd[q, n2, c]: q = f + 64*i for block n = 2*n2+i
    outd = nc.dram_tensor("out", [128, NPAIR, C], f16, kind="ExternalOutput")

    with tile.TileContext(nc) as tc:
        with (
            tc.tile_pool(name="blob_pool", bufs=1) as bp,
            tc.tile_pool(name="psum", bufs=4, space="PSUM") as pp,
            tc.tile_pool(name="osb_pool", bufs=3) as op,
        ):
            # Single input queue (sync). Stream order is NOT consumption
            # order: pair-0's blobs (h0/h1) ride BEHIND g1/g2 so the PE's
            # first matmul is gated until ~2 pairs of lookahead are
            # resident. The PE (2x faster than the stream, and penalized
            # ~2x for ~3us after any long idle by the p-state ramp) then
            # runs all 8 pairs in ONE continuous warm burst that finishes
            # right as the last group piece lands, instead of repeatedly
            # stalling cold at pair boundaries.
            # NOTE: no high_priority() here — it resets the priority
            # counter, so instructions created after the block would get
            # priorities interleaving with those inside it and the tile
            # scheduler would scramble the queue order. Plain program
            # order gives strictly monotone priorities.
            gt_tiles = {}
            gstt = bp.tile([KTAIL, GSTW], f8, tag="gst")
            nc.sync.dma_start(out=gstt[:], in_=gst[:])
            g1 = bp.tile([128, GB], f8, tag="grp1")
            nc.sync.dma_start(out=g1[:], in_=grp[0])
            gt_tiles[1] = g1
            g2 = bp.tile([128, GB], f8, tag="grp2")
            nc.sync.dma_start(out=g2[:], in_=grp[1])
            gt_tiles[2] = g2
            h0 = bp.tile([128, HB], f8, tag="h0")
            nc.sync.dma_start(out=h0[:], in_=hb[0])
            h1 = bp.tile([128, HB], f8, tag="h1")
            nc.sync.dma_start(out=h1[:], in_=hb[1])
            stt = gstt[:, : N_CHUNK * D]
            gta = gstt[:, N_CHUNK * D :]

            # Later groups stream whole (g3-g5), in halves (g6), and in
            # descending pieces (g7) so the PE's trailing granule at the
            # stream end is minimal.
            for j in range(3, NPAIR):
                assert j not in gt_tiles
                g = bp.tile([128, GB], f8, tag=f"grp{j}")
                if j == NPAIR - 1:
                    pieces = [3, 3, 1, 1]
                elif j == NPAIR - 2:
                    pieces = [4, 4]
                else:
                    pieces = [8]
                off = 0
                u = GB // 8
                for np_ in pieces:
                    nc.sync.dma_start(
                        out=g[:, off : off + np_ * u],
                        in_=grp[j - 1][:, off : off + np_ * u],
                    )
                    off += np_ * u
                gt_tiles[j] = g

            osb = None
            gs = 0
            flush_at = {3: (0, 4), 6: (4, 3), 7: (7, 1)}
            flushes = []
            for j in range(NPAIR):
                if j in (0, 4, 7):
                    osb = op.tile([128, 4, C], f16)
                    gs = j
                ps = pp.tile([128, C], f32)
                # All pairs interleave both chains (running a chain solo
                # halves PE throughput — the column groups only overlap
                # when the instruction stream alternates them). The last
                # pair runs the K-tail tile (stt/gta, long since resident)
                # FIRST so the final matmuls after the last grp piece lands
                # are plain 128-row tiles, not the slow 32-row tail. Pair 0
                # leads with chain 1 (h1, the later-arriving blob) so the
                # PE's continuous burst starts only once both are there.
                if j == NPAIR - 1:
                    korder = [KTILES - 1] + list(range(KFULL))
                    ki = [(k, i) for k in korder for i in range(2)]
                elif j == 0:
                    ki = [(k, i) for k in range(KTILES) for i in (1, 0)]
                else:
                    ki = [(k, i) for k in range(KTILES) for i in range(2)]
                kfirst, klast = ki[0][0], ki[-1][0]
                for k, i in ki:
                    n = 2 * j + i
                    if k < KFULL:
                        if j == 0:
                            blob, base = (h0 if i == 0 else h1), k * KSTRIDE
                        else:
                            blob = gt_tiles[j]
                            base = (2 * k + i) * KSTRIDE
                        lhsT = blob[:, base : base + D]
                        rhs = blob[:, base + D : base + D + C]
                    else:
                        lhsT = stt[:, n * D : n * D + D]
                        rhs = gta[:, (j * 2 + i) * C : (j * 2 + i) * C + C]
                    nc.tensor.matmul(
                        ps[64 * i : 64 * i + 64, :],
                        lhsT=lhsT,
                        rhs=rhs,
                        start=(k == kfirst),
                        stop=(k == klast),
                        tile_position=(0, 64 * i),
                    )
                scale = 1.0 / (G_SCALE * S_SCALE)
                nc.vector.tensor_scalar_mul(
                    out=osb[:, j - gs, :], in0=ps[:], scalar1=scale
                )
                if j in flush_at:
                    j0, cnt = flush_at[j]
                    flushes.append((outd[:, j0 : j0 + cnt, :], osb[:, :cnt, :]))
            # Output flushes ride the scalar (Act) HWDGE queue: issued as
            # soon as their osb tiles are ready, they overlap the input
            # stream (byte-count is conserved either way) and keep the
            # final 64 KB flush off the end of the deep sync queue.
            for dst, src in flushes:
                nc.scalar.dma_start(out=dst, in_=src)
    nc.finalize()
    return nc


def _get_compiled():
    global _compiled
    if _compiled is None:
        _compiled = _build_module()
    return _compiled


def _host_prepare(q, kv_quad_state):
    import ml_dtypes

    f8 = ml_dtypes.float8_e3m4
    qc = (q[0].astype(np.float32) * (D ** -0.5)).reshape(H, N_CHUNK, C, D)
    kv = kv_quad_state[0].astype(np.float32)  # (H, N, D, D, D)
    in_maps = []
    for h in range(H):
        # --- G2 (moving operand, transposed to K-major) ---
        G = qc[h][:, :, _iu] * qc[h][:, :, _ju]          # (N, C, PAIRS)
        G *= 0.5 * G_SCALE
        G8 = np.clip(G, -F8_MAX, F8_MAX).astype(f8)
        # [n, c, kk, pp] -> [n, pp, kk, c]
        gt_dev = (
            G8[:, :, : KFULL * 128]
            .reshape(N_CHUNK, C, KFULL, 128)
            .transpose(0, 3, 2, 1)
        )
        # tail pairs 2048+: [n, c, pp] -> [pp, (j, i, c)]
        gtta_dev = np.ascontiguousarray(
            G8[:, :, KFULL * 128 :].reshape(NPAIR, 2, C, KTAIL).transpose(3, 0, 1, 2)
        ).reshape(KTAIL, NPAIR * 2 * C)
        # --- Ssym (stationary operand, fp8 e3m4 at x2) ---
        Sh = kv[h]                                        # (N, D, D, D)
        Ss = (Sh[:, _iu, _ju, :] + Sh[:, _ju, _iu, :]) * (
            _wsym[None, :, None] * S_SCALE
        )
        Ss8 = np.clip(Ss, -F8_MAX, F8_MAX).astype(f8)     # (N, PAIRS, D)
        # [n, kk, pp, f] -> [n, pp, kk, f]
        ss_dev = (
            Ss8[:, : KFULL * 128, :]
            .reshape(N_CHUNK, KFULL, 128, D)
            .transpose(0, 2, 1, 3)
        )
        # --- blobs: per-partition cells [Ssym_k (64) | G2_k (256)] ---
        cells = np.concatenate([ss_dev, gt_dev], axis=3)  # (N, 128, KFULL, 320)
        hb_dev = cells[:2].reshape(2, 128, HB).transpose(1, 0, 2).reshape(128, 2 * HB)
        # groups: K-major cell pairs [i0_k | i1_k] so K-range DMA splits
        # feed both matmul chains
        grp_dev = (
            cells[2:].reshape(NPAIR - 1, 2, 128, KFULL, KSTRIDE)
            .transpose(2, 0, 3, 1, 4)
            .reshape(128, (NPAIR - 1) * GB)
        )
        blob_dev = np.ascontiguousarray(np.concatenate([hb_dev, grp_dev], axis=1))
        # tail: [n, pp, f] -> [pp, (n, f)]
        sst_dev = np.ascontiguousarray(
            Ss8[:, KFULL * 128 :, :].transpose(1, 0, 2)
        ).reshape(KTAIL, N_CHUNK * D)
        gst_dev = np.concatenate([sst_dev, gtta_dev], axis=1)
        in_maps.append(
            {
                "blob": blob_dev,
                "gst": gst_dev,
            }
        )
    return in_maps


def kernel(q, kv_quad_state, _trace=False):
    from concourse.bass_utils import run_bass_kernel_spmd

    nc = _get_compiled()
    in_maps = _host_prepare(q, kv_quad_state)
    res = run_bass_kernel_spmd(nc, in_maps, core_ids=list(range(N_CORES)), trace=_trace)
    out = np.empty((B, H, S_LEN, D), dtype=np.float32)
    for h in range(H):
        o = res.results[h]["out"].astype(np.float32)      # (128, 8, 256)
        # o[f + 64*i, j, c] = out[block 2j+i, c, f]
        oo = o.reshape(2, D, NPAIR, C).transpose(2, 0, 3, 1)  # (j, i, c, f)
        out[0, h] = oo.reshape(S_LEN, D)
    if _trace:
        kernel.last_exec_time_ns = res.exec_time_ns
        kernel.last_results = res
    return out

